# revision 34
# baseline (speedup 1.0000x reference)
"""Distributed 3-layer GAT on 8 Trainium2 NeuronCores (Bass/Tile), v4.

Sharding: edges sharded by (sorted) dst across 8 cores; each core produces a
contiguous shard of each layer's output nodes.

Layer 0 streams host-expanded x[src] per edge (bf16); the dense transform
(x_e @ W0p, fused with the el attention projection), edge softmax and
aggregation run per dst tile, and the next layer's table row (relu(o) @ W1p,
fused with el/er projections) is produced in the epilogue.  The layer-1 and
layer-2 tables are AllGathered (one collective each) into Shared DRAM and the
edge phases fetch per-edge rows with int16 `dma_gather` ops — the table is
split into <=32768-row subtables, each (tile-group, subtable) is one gather —
plus one tiny [P,1] indirect gather per tile for the dst-side er values.

Per dst tile the edge softmax + aggregation uses one-hot selection matmuls
accumulating in PSUM:
    S2[p, r, k] = (r == dst_local[p, k])     (k innermost -> DVE 2x bf16)
    [s|o] = sum_k S2[:,:,k]^T @ [a_k | a_k*h_k]
    out   = o * (1/(s+eps))
and the per-edge er term is expanded from the per-dst er row with a second
one-hot S3[r, (k,lane)] (built from a partition-broadcast DMA of dst_local)
via tiny [128r,128]x[128r,nh] matmuls.  Msg features are stored d-major
((d,h), h innermost; weights permuted on the host) so the per-edge a*h
multiply also hits the DVE 2x mode.  Exact because edge-softmax
normalization commutes with the segment sum; stable-softmax max-subtraction
is skipped (e = lrelu(el+er) is O(10)).  Compute/tables are bf16 (tolerance
2e-2); accumulation stays fp32 in PSUM.
"""
import os
import sys

for _p in ("/opt/trn_rl_repo", "/root/.axon_site/_ro/trn_rl_repo"):
    if os.path.isdir(_p) and _p not in sys.path:
        sys.path.insert(0, _p)

import numpy as np
import ml_dtypes

import concourse.bass as bass
import concourse.bacc as bacc
import concourse.mybir as mybir
import concourse.tile as tile
from concourse.bass_utils import run_bass_kernel_spmd
from concourse.masks import make_identity

P = 128
NCORES = 8
N0, N1, N2, N3 = 200000, 100000, 50000, 25000
FIN, F, H, D, C = 256, 192, 3, 64, 40
NEG = 0.2
EPS = 1e-30

S1, S2, S3 = N1 // NCORES, N2 // NCORES, N3 // NCORES        # 12500, 6250, 3125
T1, T2, T3 = -(-S1 // P), -(-S2 // P), -(-S3 // P)           # 98, 49, 25
PS1, PS2 = T1 * P, T2 * P                                    # 12544, 6272
ROW0 = 196                        # E0 dense out: [msg 0:192|el 192:195|pad]
ROW1 = 256                        # tab1: [msg 0:192|el 192:195|0|er 196:199|0]
ROW2 = 128                        # tab2: [msg 0:40|el 40|0|er 44|0...]
SUB = 25088                       # subtable rows (= 2*PS1 = 4*PS2), < 32768
PAD_DTL = 999.0
GT = 2                            # dst tiles per gather group (layers 1/2)
USE_DG = True                     # dma_gather (True) vs native indirect (False)

f32 = mybir.dt.float32
bf16 = mybir.dt.bfloat16
i32 = mybir.dt.int32
i16 = mybir.dt.int16
AF = mybir.ActivationFunctionType
OP = mybir.AluOpType
BF = ml_dtypes.bfloat16

# msg feature permutation: new col (d*H + h) <- old col (h*D + d)
_PERM = np.arange(F).reshape(H, D).T.reshape(-1)


# ---------------------------------------------------------------- host side --


def _core_edges(src, dst, shard_size, c):
    lo = np.searchsorted(dst, c * shard_size, side="left")
    hi = np.searchsorted(dst, (c + 1) * shard_size, side="left")
    return (src[lo:hi].astype(np.int64),
            dst[lo:hi].astype(np.int64) - c * shard_size)


def _layout(src, dst_local, n_tiles, ncpt):
    """Scatter a core's (dst-sorted) edges into the [n_tiles, ncpt, 128] slot
    grid (layer-0 streaming layout)."""
    n_slots = n_tiles * ncpt * P
    slot_src = np.zeros(n_slots, np.int64)
    slot_dtl = np.full(n_slots, PAD_DTL, np.float32)
    slot_dl = np.zeros(n_slots, np.int64)
    tile_of = dst_local // P
    tile_start = np.searchsorted(tile_of, np.arange(n_tiles), side="left")
    rank = np.arange(len(dst_local)) - tile_start[tile_of]
    pos = (tile_of * ncpt + rank // P) * P + (rank % P)
    slot_src[pos] = src
    slot_dtl[pos] = (dst_local - tile_of * P).astype(np.float32)
    slot_dl[pos] = dst_local
    return slot_src, slot_dtl, slot_dl


def _max_ncpt(src, dst, shard_size, n_tiles):
    m = 1
    for c in range(NCORES):
        _, d = _core_edges(src, dst, shard_size, c)
        cnt = np.bincount(d // P, minlength=n_tiles)
        m = max(m, int(-(-np.maximum(cnt, 1).max() // P)))
    return m


def _lane_major(a, ncols):
    return np.ascontiguousarray(np.asarray(a).reshape(ncols, P).T)


def _tab_row(g, shard, padded_shard):
    g = np.asarray(g, np.int64)
    return (g // shard * padded_shard + g % shard).astype(np.int64)


def _fuse(W, al, ar, nh, nd):
    Wel = np.einsum("khd,hd->kh", W.reshape(-1, nh, nd), al)
    Wer = np.einsum("khd,hd->kh", W.reshape(-1, nh, nd), ar)
    return np.concatenate([W, Wel, Wer], 1).astype(np.float32)


def _wstack(Wf, ncols):
    K = Wf.shape[0]
    out = np.zeros((P, 2, ncols), BF)
    out[:, 0, :] = Wf[0:P, :ncols].astype(BF)
    out[:K - P, 1, :] = Wf[P:P + (K - P), :ncols].astype(BF)
    return out


def _build_gather_meta(src, dst, T, S_cur, S_prev, PS_prev, nst):
    """Static (shared across cores) chunk layout for one layer."""
    seg_chunks = np.zeros((T, nst), np.int64)
    for c in range(NCORES):
        s, d = _core_edges(src, dst, S_cur, c)
        rows = _tab_row(s, S_prev, PS_prev)
        st = rows // SUB
        t = d // P
        cnt = np.zeros((T, nst), np.int64)
        np.add.at(cnt, (t, st), 1)
        seg_chunks = np.maximum(seg_chunks, -(-cnt // P))
    groups = []
    chunk_cursor = 0
    col_cursor = 0
    for g0 in range(0, T, GT):
        tiles = list(range(g0, min(g0 + GT, T)))
        ops = []
        tile_chunks = {t: [] for t in tiles}
        nck = 0
        MAXCH = 6  # cap descriptors per gather op (SWDGE ring is 1024 descs)
        segs = []
        for st in range(nst):
            nch = int(sum(seg_chunks[t, st] for t in tiles))
            if nch == 0:
                continue
            chunk0 = nck
            for t in tiles:
                nseg = int(seg_chunks[t, st])
                if nseg:
                    segs.append(dict(st=st, t=t, chunk0=nck, nchunks=nseg))
                for _ in range(nseg):
                    tile_chunks[t].append(nck)
                    nck += 1
            done = 0
            while done < nch:
                take = min(MAXCH, nch - done)
                ops.append(dict(st=st, col0=col_cursor, num=take * P,
                                chunk0=chunk0 + done, nchunks=take))
                col_cursor += take * P // 16
                done += take
        groups.append(dict(tiles=tiles, ops=ops, nck=nck, segs=segs,
                           tile_chunks={t: tile_chunks[t] for t in tiles},
                           chunk0_global=chunk_cursor))
        chunk_cursor += nck
    nckg = max(g["nck"] for g in groups)
    return dict(seg_chunks=seg_chunks, groups=groups, nchunks=chunk_cursor,
                cols=col_cursor, nckg=nckg)


def _build_gather_core(gm, src, dst, c, T, S_cur, S_prev, PS_prev, nst):
    """Per-core arrays for one layer's gather phase."""
    s, d = _core_edges(src, dst, S_cur, c)
    rows = _tab_row(s, S_prev, PS_prev)
    st_of = rows // SUB
    loc = rows % SUB
    t_of = d // P
    dtl_v = (d - t_of * P).astype(np.float32)

    idx16 = np.zeros((16, gm["cols"]), np.int16)
    dtl = np.full((P, gm["nchunks"]), PAD_DTL, np.float32)
    nckg = gm["nckg"]
    ngroups = len(gm["groups"])
    dtlT = np.full((ngroups, nckg * P), PAD_DTL, np.float32)
    # (replicated across partitions at the end -> [ngroups*P, nckg*P])

    order = np.lexsort((np.arange(len(s)), st_of, t_of))  # (t, st, stable)
    s_t, s_st, s_loc, s_dtl = t_of[order], st_of[order], loc[order], dtl_v[order]
    for gi, g in enumerate(gm["groups"]):
        for seg in g["segs"]:
            sel = (s_t == seg["t"]) & (s_st == seg["st"])
            lv = s_loc[sel]
            dv = s_dtl[sel]
            nseg = seg["nchunks"] * P
            assert len(lv) <= nseg
            li = np.zeros(nseg, np.int64)
            di = np.full(nseg, PAD_DTL, np.float32)
            li[:len(lv)] = lv
            di[:len(dv)] = dv
            i = np.arange(nseg)
            ch = seg["chunk0"] + i // P                   # group-chunk index
            gch = g["chunk0_global"] + ch                 # layer-chunk index
            # idx column for layer-chunk gch: gch*8 + lane//16 (ops advance
            # their col0 in chunk order, 8 cols per chunk)
            idx16[i % 16, gch * (P // 16) + (i % P) // 16] = li.astype(np.int16)
            dtl[i % P, gch] = di
            dtlT[gi, ch * P + i % P] = di
    dtlT_rep = np.ascontiguousarray(
        np.repeat(dtlT[:, None, :], P, axis=1).reshape(ngroups * P, nckg * P))
    # idx lists are read per 16-partition group by the Q7 cores: replicate
    idx16 = np.tile(idx16, (NCORES, 1))
    # per-chunk global table rows, lane-major (for the native indirect path)
    sgi = np.zeros((P, gm["nchunks"]), np.int32)
    for gi, g in enumerate(gm["groups"]):
        for seg in g["segs"]:
            i = np.arange(seg["nchunks"] * P)
            gch = g["chunk0_global"] + seg["chunk0"] + i // P
            col = gch * (P // 16) + (i % P) // 16
            sgi[i % P, gch] = (idx16[i % 16, col].astype(np.int32)
                               + seg["st"] * SUB)
    return idx16, dtl, dtlT_rep, sgi


def preprocess(inputs):
    x = np.asarray(inputs["x"], np.float32)
    src0 = np.asarray(inputs["src0"]); dst0 = np.asarray(inputs["dst0"])
    src1 = np.asarray(inputs["src1"]); dst1 = np.asarray(inputs["dst1"])
    src2 = np.asarray(inputs["src2"]); dst2 = np.asarray(inputs["dst2"])
    W0 = np.asarray(inputs["W0"], np.float32)
    al0 = np.asarray(inputs["al0"], np.float32); ar0 = np.asarray(inputs["ar0"], np.float32)
    W1 = np.asarray(inputs["W1"], np.float32)
    al1 = np.asarray(inputs["al1"], np.float32); ar1 = np.asarray(inputs["ar1"], np.float32)
    W2 = np.asarray(inputs["W2"], np.float32)
    al2 = np.asarray(inputs["al2"], np.float32); ar2 = np.asarray(inputs["ar2"], np.float32)

    W0f = _fuse(W0, al0, ar0, H, D)            # [256, 198]
    W1f = _fuse(W1, al1, ar1, H, D)            # [192, 198]
    W2f = _fuse(W2, al2, ar2, 1, C)            # [192, 42]

    W0p = np.zeros((FIN, ROW0), np.float32)
    W0p[:, 0:F] = W0f[:, _PERM]
    W0p[:, F:F + H] = W0f[:, F:F + H]
    W1p = np.zeros((F, ROW1), np.float32)
    W1p[:, 0:F] = W1f[:, _PERM]
    W1p[:, F:F + H] = W1f[:, F:F + H]
    W1p[:, 196:199] = W1f[:, F + H:F + 2 * H]
    W1p = W1p[_PERM, :]
    W2p = np.zeros((F, ROW2), np.float32)
    W2p[:, 0:C] = W2f[:, 0:C]
    W2p[:, C] = W2f[:, C]
    W2p[:, 44] = W2f[:, C + 1]
    W2p = W2p[_PERM, :]
    w0_t = _wstack(W0p, ROW0)
    w1_t = _wstack(W1p, ROW1)
    w2_t = _wstack(W2p, ROW2)

    er0_all = (x[:N1] @ W0f[:, F + H:F + 2 * H]).astype(BF)   # [N1, 3]
    xb = x.astype(BF)

    ncpt0 = _max_ncpt(src0, dst0, S1, T1)
    ncpt0 += ncpt0 % 2                         # E0 processes chunk pairs
    nc0 = T1 * ncpt0
    gm1 = _build_gather_meta(src1, dst1, T2, S2, S1, PS1, 4)
    gm2 = _build_gather_meta(src2, dst2, T3, S3, S2, PS2, 2)
    meta = dict(ncpt0=ncpt0, gm1=gm1, gm2=gm2)

    in_maps = []
    for c in range(NCORES):
        m = {}
        # ---- L0: per-edge expanded x (bf16, feature-major slabs) ----------
        s, d = _core_edges(src0, dst0, S1, c)
        e_src, e_dtl, e_dl = _layout(s, d, T1, ncpt0)
        xe5 = xb[e_src].reshape(nc0, P, 2, P)
        m["xeT"] = np.ascontiguousarray(xe5.transpose(3, 0, 2, 1))
        m["dtl0"] = _lane_major(e_dtl, nc0).astype(BF)
        erp = er0_all[(np.int64(c) * S1 + e_dl) * (e_dtl != PAD_DTL)]
        m["erp0"] = np.ascontiguousarray(
            erp.reshape(nc0, P, H).transpose(1, 0, 2)).reshape(P, nc0 * H)
        # ---- L1 ------------------------------------------------------------
        i16a, dtl, dtlT, sgi = _build_gather_core(gm1, src1, dst1, c, T2, S2, S1, PS1, 4)
        m["gx1"] = i16a
        m["dtl1"] = dtl.astype(BF)
        m["dtlT1"] = dtlT.astype(BF)
        m["sgi1"] = sgi
        g1 = np.minimum(np.int64(c) * S2 + np.arange(T2 * P), N2 - 1)
        m["erw1"] = _lane_major(_tab_row(g1, S1, PS1).astype(np.int32), T2)
        # ---- L2 ------------------------------------------------------------
        i16a, dtl, dtlT, sgi = _build_gather_core(gm2, src2, dst2, c, T3, S3, S2, PS2, 2)
        m["gx2"] = i16a
        m["dtl2"] = dtl.astype(BF)
        m["dtlT2"] = dtlT.astype(BF)
        m["sgi2"] = sgi
        g2 = np.minimum(np.int64(c) * S3 + np.arange(T3 * P), N3 - 1)
        m["erw2"] = _lane_major(_tab_row(g2, S2, PS2).astype(np.int32), T3)
        # ---- weights -------------------------------------------------------
        m["w0"] = w0_t
        m["w1"] = w1_t
        m["w2"] = w2_t
        in_maps.append(m)
    return in_maps, meta


# -------------------------------------------------------------- device side --


def build_program(meta, sim_local=False, stop_after=None):
    nc = _build_body(meta, sim_local, stop_after)
    nc.finalize()
    return nc


def _build_body(meta, sim_local=False, stop_after=None):
    ncpt0 = meta["ncpt0"]
    nc0 = T1 * ncpt0
    gm1, gm2 = meta["gm1"], meta["gm2"]
    kmax = max(ncpt0, gm1["nckg"], gm2["nckg"])

    nc = bacc.Bacc("TRN2", target_bir_lowering=False, debug=False,
                   num_devices=NCORES)
    xeT = nc.declare_dram_parameter("xeT", [P, nc0, 2, P], bf16, isOutput=False)
    dtl0 = nc.declare_dram_parameter("dtl0", [P, nc0], bf16, isOutput=False)
    erp0 = nc.declare_dram_parameter("erp0", [P, nc0 * H], bf16, isOutput=False)
    gx1 = nc.declare_dram_parameter("gx1", [P, gm1["cols"]], i16, isOutput=False)
    sgi1 = nc.declare_dram_parameter("sgi1", [P, gm1["nchunks"]], i32, isOutput=False)
    dtl1 = nc.declare_dram_parameter("dtl1", [P, gm1["nchunks"]], bf16, isOutput=False)
    dtlT1 = nc.declare_dram_parameter("dtlT1", [len(gm1["groups"]) * P, gm1["nckg"] * P],
                                      bf16, isOutput=False)
    erw1 = nc.declare_dram_parameter("erw1", [P, T2], i32, isOutput=False)
    gx2 = nc.declare_dram_parameter("gx2", [P, gm2["cols"]], i16, isOutput=False)
    sgi2 = nc.declare_dram_parameter("sgi2", [P, gm2["nchunks"]], i32, isOutput=False)
    dtl2 = nc.declare_dram_parameter("dtl2", [P, gm2["nchunks"]], bf16, isOutput=False)
    dtlT2 = nc.declare_dram_parameter("dtlT2", [len(gm2["groups"]) * P, gm2["nckg"] * P],
                                      bf16, isOutput=False)
    erw2 = nc.declare_dram_parameter("erw2", [P, T3], i32, isOutput=False)
    w0 = nc.declare_dram_parameter("w0", [P, 2, ROW0], bf16, isOutput=False)
    w1 = nc.declare_dram_parameter("w1", [P, 2, ROW1], bf16, isOutput=False)
    w2 = nc.declare_dram_parameter("w2", [P, 2, ROW2], bf16, isOutput=False)
    out = nc.declare_dram_parameter("out", [S3, C], f32, isOutput=True)

    def _ag(in_ap, out_ap):
        if sim_local:
            n = in_ap.shape[0]
            for r in range(NCORES):
                nc.sync.dma_start(out=out_ap[r * n:(r + 1) * n, :], in_=in_ap)
        else:
            nc.gpsimd.collective_compute(
                "AllGather", OP.bypass,
                replica_groups=[list(range(NCORES))],
                ins=[in_ap], outs=[out_ap],
            )

    with tile.TileContext(nc) as tc:
        with (
            tc.tile_pool(name="cst", bufs=1) as cst,
            tc.tile_pool(name="sb", bufs=2) as sb,
            tc.tile_pool(name="ps", bufs=2, space="PSUM") as ps,
            tc.tile_pool(name="dram", bufs=1, space="DRAM") as dram,
        ):
            # ---- constants / resident tensors ---------------------------
            ident = cst.tile([P, P], bf16)
            make_identity(nc, ident[:])
            iota_i = cst.tile([P, P], i32)
            nc.gpsimd.iota(iota_i[:], pattern=[[1, P]], base=0, channel_multiplier=0)
            iota_b = cst.tile([P, P], bf16)
            nc.vector.tensor_copy(iota_b[:], iota_i[:])
            iota_mid = cst.tile([P, P, kmax], bf16)
            nc.vector.tensor_copy(
                iota_mid[:], iota_b[:, :, None].broadcast_to([P, P, kmax]))
            iotp_i = cst.tile([P, 1], i32)
            nc.gpsimd.iota(iotp_i[:], pattern=[[0, 1]], base=0, channel_multiplier=1)
            iota_part = cst.tile([P, 1], f32)
            nc.vector.tensor_copy(iota_part[:], iotp_i[:])

            w0_t = cst.tile([P, 2, ROW0], bf16)
            nc.sync.dma_start(w0_t[:], w0[:])
            w1_t = cst.tile([P, 2, ROW1], bf16)
            nc.sync.dma_start(w1_t[:], w1[:])
            w2_t = cst.tile([P, 2, ROW2], bf16)
            nc.sync.dma_start(w2_t[:], w2[:])

            dtl0_t = cst.tile([P, nc0], bf16)
            nc.sync.dma_start(dtl0_t[:], dtl0[:])
            erp0_t = cst.tile([P, nc0, H], bf16)
            nc.sync.dma_start(erp0_t[:].rearrange("p k h -> p (k h)"), erp0[:])
            gx1_t = cst.tile([P, gm1["cols"]], i16)
            nc.sync.dma_start(gx1_t[:], gx1[:])
            sgi1_t = cst.tile([P, gm1["nchunks"]], i32)
            nc.sync.dma_start(sgi1_t[:], sgi1[:])
            dtl1_t = cst.tile([P, gm1["nchunks"]], bf16)
            nc.sync.dma_start(dtl1_t[:], dtl1[:])
            erw1_t = cst.tile([P, T2], i32)
            nc.sync.dma_start(erw1_t[:], erw1[:])
            gx2_t = cst.tile([P, gm2["cols"]], i16)
            nc.sync.dma_start(gx2_t[:], gx2[:])
            sgi2_t = cst.tile([P, gm2["nchunks"]], i32)
            nc.sync.dma_start(sgi2_t[:], sgi2[:])
            dtl2_t = cst.tile([P, gm2["nchunks"]], bf16)
            nc.sync.dma_start(dtl2_t[:], dtl2[:])
            erw2_t = cst.tile([P, T3], i32)
            nc.sync.dma_start(erw2_t[:], erw2[:])

            tab1_sh = dram.tile([PS1, ROW1], bf16, name="t1s")
            tab2_sh = dram.tile([PS2, ROW2], bf16, name="t2s")
            ag_space = {} if sim_local else {"addr_space": "Shared"}
            tab1_ag = dram.tile([NCORES * PS1, ROW1], bf16, name="t1a", **ag_space)
            tab2_ag = dram.tile([NCORES * PS2, ROW2], bf16, name="t2a", **ag_space)

            # ---------------- shared epilogue helpers --------------------
            def _norm(so_ps, nh, nf):
                r_t = sb.tile([P, nh], f32, tag="r")
                nc.vector.tensor_scalar(out=r_t[:], in0=so_ps[:, 0:nh],
                                        scalar1=EPS, scalar2=None, op0=OP.add)
                nc.vector.reciprocal(r_t[:], r_t[:])
                o_nb = sb.tile([P, nf], bf16, tag="onb")
                if nh > 1:
                    nc.vector.tensor_tensor(
                        out=o_nb[:].rearrange("p (d h) -> p d h", h=nh),
                        in0=so_ps[:, nh:].rearrange("p (d h) -> p d h", h=nh),
                        in1=r_t[:, None, :].broadcast_to([P, nf // nh, nh]),
                        op=OP.mult,
                    )
                else:
                    nc.vector.tensor_tensor(
                        out=o_nb[:], in0=so_ps[:, nh:],
                        in1=r_t[:, 0:nh].broadcast_to([P, nf]), op=OP.mult)
                return o_nb

            def _fused_next(o_nb, w_t, ncols):
                tp_a = ps.tile([P, P], bf16, tag="tpa", bufs=1)
                nc.tensor.transpose(tp_a[:], o_nb[:, 0:P], ident[:])
                oT_a = sb.tile([P, P], bf16, tag="oTa")
                nc.scalar.activation(out=oT_a[:], in_=tp_a[:], func=AF.Relu)
                tp_b = ps.tile([F - P, P], bf16, tag="tpb", bufs=1)
                nc.tensor.transpose(tp_b[:], o_nb[:, P:F], ident[:])
                oT_b = sb.tile([F - P, P], bf16, tag="oTb")
                nc.scalar.activation(out=oT_b[:], in_=tp_b[:], func=AF.Relu)
                t_ps = ps.tile([P, ncols], f32, tag="tps", bufs=1)
                nc.tensor.matmul(out=t_ps[:], lhsT=oT_a[:], rhs=w_t[:, 0, :ncols],
                                 start=True, stop=False)
                nc.tensor.matmul(out=t_ps[:], lhsT=oT_b[:], rhs=w_t[:F - P, 1, :ncols],
                                 start=False, stop=True)
                t_sb = sb.tile([P, ncols], bf16, tag="tsb")
                nc.scalar.copy(t_sb[:], t_ps[:])
                return t_sb

            # ---- phase E0: layer-0 dense + edge + fused-W1 --------------
            for t in range(T1):
                xe = sb.tile([P, ncpt0, 2, P], bf16, tag="xe", bufs=3)
                nc.sync.dma_start(xe[:], xeT[:, t * ncpt0:(t + 1) * ncpt0, :, :])
                h_sb = sb.tile([P, ncpt0, ROW0], bf16, tag="h")
                for j in range(ncpt0 // 2):
                    h_ps = ps.tile([P, 2, ROW0], f32, tag="hps")
                    for jj in range(2):
                        k = 2 * j + jj
                        for kk in range(2):
                            nc.tensor.matmul(out=h_ps[:, jj, :],
                                             lhsT=xe[:, k, kk, :],
                                             rhs=w0_t[:, kk, :],
                                             start=(kk == 0), stop=(kk == 1))
                    if j % 2 == 0:
                        nc.scalar.copy(h_sb[:, 2 * j:2 * j + 2, :], h_ps[:])
                    else:
                        nc.vector.tensor_copy(h_sb[:, 2 * j:2 * j + 2, :], h_ps[:])
                s2 = sb.tile([P, P, ncpt0], bf16, tag="s2")
                nc.vector.tensor_tensor(
                    out=s2[:], in0=iota_mid[:, :, 0:ncpt0],
                    in1=dtl0_t[:, None, t * ncpt0:(t + 1) * ncpt0]
                        .broadcast_to([P, P, ncpt0]),
                    op=OP.is_equal)
                am = sb.tile([P, ncpt0, H + F], bf16, tag="am")
                a_t = sb.tile([P, ncpt0, H], bf16, tag="a")
                nc.vector.tensor_tensor(
                    out=a_t[:], in0=h_sb[:, :, F:F + H],
                    in1=erp0_t[:, t * ncpt0:(t + 1) * ncpt0, :], op=OP.add)
                nc.vector.scalar_tensor_tensor(
                    out=a_t[:], in0=a_t[:], scalar=NEG, in1=a_t[:],
                    op0=OP.mult, op1=OP.max)
                nc.scalar.activation(out=am[:, :, 0:H], in_=a_t[:], func=AF.Exp)
                nc.vector.tensor_tensor(
                    out=am[:, :, H:].rearrange("p k (d h) -> p k d h", h=H),
                    in0=h_sb[:, :, 0:F].rearrange("p k (d h) -> p k d h", h=H),
                    in1=am[:, :, None, 0:H].broadcast_to([P, ncpt0, D, H]),
                    op=OP.mult)
                so_ps = ps.tile([P, H + F], f32, tag="so")
                for k in range(ncpt0):
                    nc.tensor.matmul(out=so_ps[:], lhsT=s2[:, :, k],
                                     rhs=am[:, k, :],
                                     start=(k == 0), stop=(k == ncpt0 - 1))
                o_nb = _norm(so_ps, H, F)
                t_sb = _fused_next(o_nb, w1_t, ROW1)
                nc.sync.dma_start(out=tab1_sh[t * P:(t + 1) * P, :], in_=t_sb[:])
            _ag(tab1_sh[:], tab1_ag[:])

            # ---- gather-based edge phase (layers 1, 2) ------------------
            def edge_phase(gm, tab_ag, row, gx_t, dtl_t, dtlT_p, erw_t,
                           el_off, er_off, nh, nf, epilogue, gather_only=False,
                           sgi_t=None):
                for gi, g in enumerate(gm["groups"]):
                    nck = g["nck"]
                    hg = sb.tile([P, gm["nckg"], row], bf16, tag="hg", bufs=3)
                    if USE_DG:
                        for op in g["ops"]:
                            nc.gpsimd.dma_gather(
                                out_ap=hg[:, op["chunk0"]:op["chunk0"] + op["nchunks"], :],
                                in_ap=tab_ag[op["st"] * SUB:(op["st"] + 1) * SUB, :],
                                idxs_ap=gx_t[:, op["col0"]:op["col0"] + op["num"] // 16],
                                num_idxs=op["num"],
                                num_idxs_reg=op["num"],
                                elem_size=row,
                            )
                    else:
                        c0g = g["chunk0_global"]
                        for k in range(nck):
                            nc.gpsimd.indirect_dma_start(
                                out=hg[:, k, :], out_offset=None, in_=tab_ag[:],
                                in_offset=bass.IndirectOffsetOnAxis(
                                    ap=sgi_t[:, c0g + k:c0g + k + 1], axis=0),
                            )
                    # dst-side er rows, one [P,1] indirect gather per tile
                    erw = sb.tile([P, GT, 4], bf16, tag="erw", bufs=3)
                    for j, t in enumerate(g["tiles"]):
                        nc.gpsimd.indirect_dma_start(
                            out=erw[:, j, :], out_offset=None, in_=tab_ag[:],
                            in_offset=bass.IndirectOffsetOnAxis(
                                ap=erw_t[:, t:t + 1], axis=0),
                            element_offset=er_off,
                        )
                    if gather_only:
                        continue
                    # per-edge er via one-hot S3 (r on partitions)
                    dtlrep = sb.tile([P, gm["nckg"] * P], bf16, tag="dtlrep", bufs=3)
                    nc.sync.dma_start(
                        out=dtlrep[:, 0:nck * P],
                        in_=dtlT_p[gi * P:(gi + 1) * P, 0:nck * P])
                    s3 = sb.tile([P, gm["nckg"] * P], bf16, tag="s3")
                    nc.vector.tensor_scalar(
                        out=s3[:, 0:nck * P], in0=dtlrep[:, 0:nck * P],
                        scalar1=iota_part[:, 0:1], scalar2=None,
                        op0=OP.is_equal)
                    er_ps = ps.tile([P, gm["nckg"], nh], f32, tag="erps", bufs=1)
                    ch_tile = {}
                    for t in g["tiles"]:
                        for k in g["tile_chunks"][t]:
                            ch_tile[k] = t - g["tiles"][0]
                    for k in range(nck):
                        nc.tensor.matmul(
                            out=er_ps[:, k, :],
                            lhsT=s3[:, k * P:(k + 1) * P],
                            rhs=erw[:, ch_tile[k], 0:nh],
                            start=True, stop=True)
                    # edge softmax pieces for the whole group
                    s2 = sb.tile([P, P, gm["nckg"]], bf16, tag="s2")
                    c0 = g["chunk0_global"]
                    nc.vector.tensor_tensor(
                        out=s2[:, :, 0:nck], in0=iota_mid[:, :, 0:nck],
                        in1=dtl_t[:, None, c0:c0 + nck]
                            .broadcast_to([P, P, nck]),
                        op=OP.is_equal)
                    am = sb.tile([P, gm["nckg"], nh + nf], bf16, tag="am")
                    a_t = sb.tile([P, gm["nckg"], nh], bf16, tag="a")
                    nc.vector.tensor_tensor(
                        out=a_t[:, 0:nck, :], in0=hg[:, 0:nck, el_off:el_off + nh],
                        in1=er_ps[:, 0:nck, :], op=OP.add)
                    nc.vector.scalar_tensor_tensor(
                        out=a_t[:, 0:nck, :], in0=a_t[:, 0:nck, :], scalar=NEG,
                        in1=a_t[:, 0:nck, :], op0=OP.mult, op1=OP.max)
                    nc.scalar.activation(out=am[:, 0:nck, 0:nh],
                                         in_=a_t[:, 0:nck, :], func=AF.Exp)
                    if nh > 1:
                        nc.vector.tensor_tensor(
                            out=am[:, 0:nck, nh:].rearrange(
                                "p k (d h) -> p k d h", h=nh),
                            in0=hg[:, 0:nck, 0:nf].rearrange(
                                "p k (d h) -> p k d h", h=nh),
                            in1=am[:, 0:nck, None, 0:nh]
                                .broadcast_to([P, nck, nf // nh, nh]),
                            op=OP.mult)
                    else:
                        nc.vector.tensor_tensor(
                            out=am[:, 0:nck, nh:], in0=hg[:, 0:nck, 0:nf],
                            in1=am[:, 0:nck, 0:nh].broadcast_to([P, nck, nf]),
                            op=OP.mult)
                    for t in g["tiles"]:
                        ks = g["tile_chunks"][t]
                        if not ks:
                            continue
                        so_ps = ps.tile([P, nh + nf], f32, tag="so")
                        for i, k in enumerate(ks):
                            nc.tensor.matmul(out=so_ps[:], lhsT=s2[:, :, k],
                                             rhs=am[:, k, :],
                                             start=(i == 0),
                                             stop=(i == len(ks) - 1))
                        epilogue(t, so_ps)

            def _dummy_out():
                z = sb.tile([P, C], f32, tag="of")
                nc.vector.memset(z[:], 0.0)
                for t in range(T3):
                    rows = min(P, S3 - t * P)
                    nc.sync.dma_start(out=out[t * P:t * P + rows, :],
                                      in_=z[:rows, :])

            if stop_after == "e0":
                _dummy_out()
                return nc

            def epi1(t, so_ps):
                o_nb = _norm(so_ps, H, F)
                t_sb = _fused_next(o_nb, w2_t, ROW2)
                nc.sync.dma_start(out=tab2_sh[t * P:(t + 1) * P, :], in_=t_sb[:])

            edge_phase(gm1, tab1_ag, ROW1, gx1_t, dtl1_t, dtlT1, erw1_t,
                       F, 196, H, F, epi1, sgi_t=sgi1_t)
            _ag(tab2_sh[:], tab2_ag[:])

            if stop_after == "e1":
                _dummy_out()
                return nc

            def epi2(t, so_ps):
                r_t = sb.tile([P, 1], f32, tag="r2")
                nc.vector.tensor_scalar(out=r_t[:], in0=so_ps[:, 0:1],
                                        scalar1=EPS, scalar2=None, op0=OP.add)
                nc.vector.reciprocal(r_t[:], r_t[:])
                o_f = sb.tile([P, C], f32, tag="of")
                nc.vector.tensor_tensor(
                    out=o_f[:], in0=so_ps[:, 1:1 + C],
                    in1=r_t[:, 0:1].broadcast_to([P, C]), op=OP.mult)
                rows = min(P, S3 - t * P)
                nc.sync.dma_start(out=out[t * P:t * P + rows, :],
                                  in_=o_f[:rows, :])

            edge_phase(gm2, tab2_ag, ROW2, gx2_t, dtl2_t, dtlT2, erw2_t,
                       C, 44, 1, C, epi2, gather_only=(stop_after == "e2g"),
                       sgi_t=sgi2_t)
            if stop_after == "e2g":
                _dummy_out()
    return nc


_CACHE = {}
LAST_RESULT = None


def _meta_key(meta):
    import hashlib, pickle
    return hashlib.sha1(pickle.dumps(
        (meta["ncpt0"],
         meta["gm1"]["seg_chunks"].tobytes(), meta["gm1"]["cols"],
         meta["gm2"]["seg_chunks"].tobytes(), meta["gm2"]["cols"]))).hexdigest()


def kernel(**inputs):
    global LAST_RESULT
    in_maps, meta = preprocess(inputs)
    key = _meta_key(meta)
    if key not in _CACHE:
        _CACHE[key] = build_program(meta)
    nc = _CACHE[key]
    res = run_bass_kernel_spmd(nc, in_maps, core_ids=list(range(NCORES)))
    LAST_RESULT = res
    return np.concatenate(
        [np.asarray(res.results[c]["out"], np.float32) for c in range(NCORES)], 0)
